# revision 22
# baseline (speedup 1.0000x reference)
"""Trainium2 Bass kernel for CrossAttention (silu-scored, masked) sharded over
8 NeuronCores.

Problem (full shapes):
    query/key/value: [2, 2048, 1024] f32, mask: [2, 1, 2048, 2048] int32
    Wq/Wk/Wv/Wo: [1024, 1024] f32, bq/bk/bv: [1024] f32
    out = silu(mask((q @ k.T) * scale)) @ v heads-merged @ Wo.T

Sharding: core c handles batch b = c // 4 and heads 4*(c%4) .. 4*(c%4)+3
(data parallel on B, tensor parallel on heads).  Each core computes a
row-parallel partial of the O-projection; the host sums the 4 partials per
batch.  No cross-device communication.

Per-core device program (everything computed transposed so the PE contracts
on the partition dim; all matmul operands bf16, f32 PSUM accumulate):
  qT = SCALE*(Wq_loc @ x_q) + b   [256, 2048] bf16  (x pre-transposed on host)
  kT -> 4 per-head tiles [128, 2048], zero-padded to K=128 (full-array
        matmuls keep the PE HAM governor at 2.4 GHz; half-array K=64 work
        reads as ~50% activity and gets clocked down to 1.2 GHz)
  vT = Wv_loc @ x_v -> PE-transpose -> v [sk, 64|0pad] bf16 (M=128 padded)
  per head h, per sk-tile j (mask row streamed from HBM on the fly):
      sT[sk, sq]   = kt[h].T @ qT          (PSUM f32)
      attn         = silu(sT) * maskT      (ACT silu -> bf16, DVE mask mul)
      ctx acc     += v_pad.T @ attn        (PSUM, 16-step accumulate)
  partial = ctxT.T @ Wo_locT -> bf16; host sums the 4 partials per batch.
"""

import os
import numpy as np
import ml_dtypes

B = 2
S = 2048
HID = 1024
HEADS = 16
DH = 64
N_CORES = 8
GROUPS = 4          # head-groups (cores per batch)
NH_LOC = HEADS // GROUPS  # 4 heads per core
DLOC = NH_LOC * DH        # 256 local features
SCALE = DH ** -0.5

F32 = np.float32
BF16 = ml_dtypes.bfloat16

_COMPILED = {}


def build_program():
    import concourse.bass as bass
    import concourse.tile as tile
    from concourse import bacc, mybir
    from concourse.masks import make_identity

    f32 = mybir.dt.float32
    bf16 = mybir.dt.bfloat16

    nc = bacc.Bacc("TRN2", target_bir_lowering=False, debug=False,
                   enable_asserts=False, num_devices=N_CORES)

    xq = nc.dram_tensor("xq", [HID, S], bf16, kind="ExternalInput").ap()
    xk = nc.dram_tensor("xk", [HID, S], bf16, kind="ExternalInput").ap()
    xv = nc.dram_tensor("xv", [HID, S], bf16, kind="ExternalInput").ap()
    mk = nc.dram_tensor("mk", [S, S], bf16, kind="ExternalInput").ap()
    wq = nc.dram_tensor("wq", [HID, DLOC], bf16, kind="ExternalInput").ap()
    wk = nc.dram_tensor("wk", [HID, DLOC], bf16, kind="ExternalInput").ap()
    wv = nc.dram_tensor("wv", [HID, DLOC], bf16, kind="ExternalInput").ap()
    wo = nc.dram_tensor("wo", [DLOC, HID], bf16, kind="ExternalInput").ap()
    bq = nc.dram_tensor("bq", [DLOC, 1], f32, kind="ExternalInput").ap()
    bk = nc.dram_tensor("bk", [DLOC, 1], f32, kind="ExternalInput").ap()
    bv = nc.dram_tensor("bv", [DLOC, 1], f32, kind="ExternalInput").ap()
    out1 = nc.dram_tensor("out1", [S, HID], bf16, kind="ExternalOutput").ap()

    SILU = mybir.ActivationFunctionType.Silu
    MUL = mybir.AluOpType.mult
    ADD = mybir.AluOpType.add

    with tile.TileContext(nc) as tc:
        with (
            tc.tile_pool(name="res", bufs=1) as res,
            tc.tile_pool(name="io", bufs=4) as io,
            # single PSUM pool for all phases: pacc0/pacc1 (2x1 bank slots
            # each) and a 2x2-bank tag shared by the transpose scratch and
            # the score tiles.  Phase B reuses the pacc slots for its ctx
            # accumulators and phase C for its O-proj tiles, so attention
            # overlaps the projection tail without exceeding 8 banks.
            tc.tile_pool(name="ps", bufs=2, space="PSUM") as ps,
            tc.tile_pool(name="wA", bufs=2) as wA,
            tc.tile_pool(name="attp", bufs=3) as attp,
            tc.tile_pool(name="mpool", bufs=4) as mpool,
            tc.tile_pool(name="oev", bufs=4) as oev,
        ):
            # ---- resident SBUF tensors ----
            qt = [res.tile([128, S], bf16, tag=f"qt{m}", name=f"qt{m}") for m in range(2)]
            # kt/v are zero-padded to full K=128 / M=128 so phase-B matmuls
            # light up the whole PE array (HAM reads half-array matmuls as
            # ~50% activity and throttles the clock to 1.2 GHz)
            kt = [res.tile([128, S], bf16, tag=f"kt{h}", name=f"kt{h}") for h in range(NH_LOC)]
            vt_bf = [res.tile([128, S], bf16, tag=f"vt{m}", name=f"vt{m}") for m in range(2)]
            v_bf = res.tile([128, 16 * 4 * 128], bf16, tag="vbf", name="v_bf")  # [p, (j*4+h)*128 + c]
            for h in range(NH_LOC):
                nc.gpsimd.memset(kt[h][:, :], 0.0)
            nc.gpsimd.memset(v_bf[:, :], 0.0)
            ctxt = [res.tile([128, S], bf16, tag=f"ctxt{m}", name=f"ctxt{m}") for m in range(2)]
            wo_sb = [res.tile([128, HID], bf16, tag=f"wo{k}", name=f"wo_sb{k}") for k in range(2)]
            ident = res.tile([128, 128], bf16, tag="ident", name="ident")
            b_sb = {}
            for nm, srcb in (("bq", bq), ("bk", bk), ("bv", bv)):
                b_sb[nm] = [res.tile([128, 1], f32, tag=f"{nm}{m}", name=f"{nm}_sb{m}") for m in range(2)]
                for m in range(2):
                    nc.sync.dma_start(out=b_sb[nm][m][:, :], in_=srcb[m * 128:(m + 1) * 128, :])
            for k in range(2):
                nc.sync.dma_start(out=wo_sb[k][:, :], in_=wo[k * 128:(k + 1) * 128, :])
            make_identity(nc, ident[:, :])
            # dummy silu: pulls the ACT table-set load (~2.7us) into phase A
            warm = res.tile([128, 1], bf16, tag="warm", name="warm")
            nc.scalar.activation(warm[:, :], b_sb["bq"][0][:, 0:1], SILU)

            # ---- Phase A: QKV projections ----
            projs = [
                ("v", xv, wv, "bv", 1.0, vt_bf),
                ("q", xq, wq, "bq", SCALE, qt),
                ("k", xk, wk, "bk", 1.0, None),
            ]

            def v_transpose(m, j):
                # PE filler: keeps duty high while DVE drains proj evacs
                tr = ps.tile([128, 128], bf16, tag="sT", name="tr")
                nc.tensor.transpose(
                    tr[:, :], vt_bf[m][:, j * 128:(j + 1) * 128], ident[:, :]
                )
                for hh in range(2):
                    h = 2 * m + hh
                    nc.vector.tensor_copy(
                        out=v_bf[:, (j * 4 + h) * 128: (j * 4 + h) * 128 + 64],
                        in_=tr[:, hh * 64:(hh + 1) * 64],
                    )

            tr_done = 0
            pinfo = {p[0]: p for p in projs}
            loaded = {}

            def load_proj(nm):
                _, x_ap, w_ap, _, _, _ = pinfo[nm]
                w_sb = wA.tile([128, 8 * DLOC], bf16, tag="w", name=f"w_{nm}")
                nc.scalar.dma_start(
                    out=w_sb[:, :].rearrange("p (k m) -> p k m", k=8),
                    in_=w_ap.rearrange("(k p) m -> p k m", p=128),
                )
                xfull = []
                for k in range(8):
                    xt = io.tile([128, S], bf16, tag="xfull", name=f"xf_{nm}{k}", bufs=16)
                    dmae = nc.sync if k % 2 == 0 else nc.scalar
                    dmae.dma_start(out=xt[:, :], in_=x_ap[k * 128:(k + 1) * 128, :])
                    xfull.append(xt)
                loaded[nm] = (w_sb, xfull)

            # unit order: head-pair 0/1 of q and k complete after 3 of 5
            # units, so phase-B silu starts while m=1 still projects
            units = [("v", [0, 1]), ("q", [0]), ("k", [0]), ("q", [1]), ("k", [1])]
            for nm, ms in units:
                _, x_ap, w_ap, bnm, scl, dst = pinfo[nm]
                if nm not in loaded:
                    load_proj(nm)
                w_sb, xfull = loaded[nm]
                if True:
                    for n in range(4):
                        pacc = {m: ps.tile([128, 512], f32, tag=f"pacc{m}", name=f"pacc{m}") for m in ms}
                        for k in range(8):
                            for m in ms:
                                nc.tensor.matmul(
                                    pacc[m][:, :],
                                    lhsT=w_sb[:, k * DLOC + m * 128: k * DLOC + (m + 1) * 128],
                                    rhs=xfull[k][:, n * 512:(n + 1) * 512],
                                    start=(k == 0), stop=(k == 7),
                                )
                        for m in ms:
                            if nm == "k":
                                # scatter heads into their zero-padded kt tiles
                                for hh in range(2):
                                    ro = hh * 64
                                    nc.vector.tensor_scalar(
                                        out=kt[2 * m + hh][ro:ro + 64, n * 512:(n + 1) * 512],
                                        in0=pacc[m][ro:ro + 64, :],
                                        scalar1=float(scl),
                                        scalar2=b_sb[bnm][m][ro:ro + 64, 0:1],
                                        op0=MUL, op1=ADD,
                                    )
                            else:
                                nc.vector.tensor_scalar(
                                    out=dst[m][:, n * 512:(n + 1) * 512],
                                    in0=pacc[m][:, :],
                                    scalar1=float(scl),
                                    scalar2=b_sb[bnm][m][:, 0:1],
                                    op0=MUL, op1=ADD,
                                )
                        # interleave v transposes into q/k streams as PE filler
                        if nm != "v":
                            for _ in range(4):
                                if tr_done < 32:
                                    v_transpose(tr_done // 16, tr_done % 16)
                                    tr_done += 1

            # ---- Phase B: attention per head; maskT streamed from HBM ----
            for h in range(NH_LOC):
                t_i = h // 2
                po = (h % 2) * 64
                acc = [ps.tile([128, 512], f32, tag=f"pacc{s // 2}", name=f"acc{s}") for s in range(4)]
                for j in range(16):
                    mch = mpool.tile([128, S], bf16, tag="mch", name="mch")
                    nc.sync.dma_start(out=mch[:, :], in_=mk[j * 128:(j + 1) * 128, :])
                    for half in range(2):
                        sT = ps.tile([128, 1024], f32, tag="sT", name="sT")
                        for s2 in range(2):
                            sqb = half * 2 + s2
                            nc.tensor.matmul(
                                sT[:, s2 * 512:(s2 + 1) * 512],
                                lhsT=kt[h][:, j * 128:(j + 1) * 128],
                                rhs=qt[t_i][:, sqb * 512:(sqb + 1) * 512],
                                start=True, stop=True,
                            )
                        att = attp.tile([128, 1024], bf16, tag="att", name="att")
                        nc.scalar.activation(att[:, :], sT[:, :], SILU)
                        attn = attp.tile([128, 1024], bf16, tag="attn", name="attn", bufs=6)
                        nc.vector.tensor_mul(
                            out=attn[:, :],
                            in0=att[:, :],
                            in1=mch[:, half * 1024: half * 1024 + 1024],
                        )
                        for s2 in range(2):
                            sqb = half * 2 + s2
                            nc.tensor.matmul(
                                acc[sqb][:, :],
                                lhsT=v_bf[:, (j * 4 + h) * 128: (j * 4 + h + 1) * 128],
                                rhs=attn[:, s2 * 512:(s2 + 1) * 512],
                                start=(j == 0), stop=(j == 15),
                            )
                for sqb in range(4):
                    nc.vector.tensor_copy(
                        out=ctxt[t_i][po:po + 64, sqb * 512:(sqb + 1) * 512],
                        in_=acc[sqb][0:64, :],
                    )
            # ---- Phase C: O projection ----
            for mb in range(16):
                for n2 in range(2):
                    pot = ps.tile([128, 512], f32, tag=f"pacc{mb % 2}", name="pot")
                    for ki in range(2):
                        nc.tensor.matmul(
                            pot[:, :],
                            lhsT=ctxt[ki][:, mb * 128:(mb + 1) * 128],
                            rhs=wo_sb[ki][:, n2 * 512:(n2 + 1) * 512],
                            start=(ki == 0), stop=(ki == 1),
                        )
                    ev = oev.tile([128, 512], bf16, tag="oev", name="ev")
                    if mb % 2 == 0:
                        nc.vector.tensor_copy(out=ev[:, :], in_=pot[:, :])
                    else:
                        nc.scalar.copy(out=ev[:, :], in_=pot[:, :])
                    nc.sync.dma_start(
                        out=out1[mb * 128:(mb + 1) * 128, n2 * 512:(n2 + 1) * 512],
                        in_=ev[:, :],
                    )


    nc.compile()
    return nc


def get_program():
    if "nc" not in _COMPILED:
        _COMPILED["nc"] = build_program()
    return _COMPILED["nc"]


def make_in_maps(query, key, value, mask, Wq, bq, Wk, bk, Wv, bv, Wo):
    """Host-side sharding/layout prep: one input map per core."""
    query = np.asarray(query, dtype=F32)
    key = np.asarray(key, dtype=F32)
    value = np.asarray(value, dtype=F32)
    mask = np.asarray(mask)
    in_maps = []
    maskT = [np.ascontiguousarray(mask[b, 0].T).astype(BF16) for b in range(B)]
    xqT = [np.ascontiguousarray(query[b].T).astype(BF16) for b in range(B)]
    xkT = [np.ascontiguousarray(key[b].T).astype(BF16) for b in range(B)]
    xvT = [np.ascontiguousarray(value[b].T).astype(BF16) for b in range(B)]
    for c in range(N_CORES):
        b = c // GROUPS
        g = c % GROUPS
        rs = slice(g * DLOC, (g + 1) * DLOC)
        in_maps.append({
            "xq": xqT[b],
            "xk": xkT[b],
            "xv": xvT[b],
            "mk": maskT[b],
            "wq": np.ascontiguousarray(np.asarray(Wq, F32)[rs, :].T).astype(BF16),
            "wk": np.ascontiguousarray(np.asarray(Wk, F32)[rs, :].T).astype(BF16),
            "wv": np.ascontiguousarray(np.asarray(Wv, F32)[rs, :].T).astype(BF16),
            "wo": np.ascontiguousarray(np.asarray(Wo, F32)[:, rs].T).astype(BF16),
            "bq": (SCALE * np.asarray(bq, F32)[rs]).reshape(DLOC, 1),
            "bk": np.asarray(bk, F32)[rs].reshape(DLOC, 1),
            "bv": np.asarray(bv, F32)[rs].reshape(DLOC, 1),
        })
    return in_maps


def run_on_device(in_maps, trace=False, tmpdir=None):
    from concourse.bass_utils import run_bass_kernel_spmd
    nc = get_program()
    kwargs = {}
    if trace:
        kwargs.update(trace=True, tmpdir=tmpdir)
    return run_bass_kernel_spmd(nc, in_maps, list(range(N_CORES)), **kwargs)


def assemble_output(results):
    out = np.zeros((B, S, HID), dtype=F32)
    for c in range(N_CORES):
        out[c // GROUPS] += results[c]["out1"].astype(F32)
    return out


def kernel(query, key, value, mask, Wq, bq, Wk, bk, Wv, bv, Wo):
    in_maps = make_in_maps(query, key, value, mask, Wq, bq, Wk, bk, Wv, bv, Wo)
    res = run_on_device(in_maps)
    return assemble_output(res.results)
